# revision 9
# baseline (speedup 1.0000x reference)
"""CRNN (im2col conv patches -> 3-layer stacked LSTM) Trainium2 kernel.

Strategy: data-parallel over batch (B=32 -> 4 rows/core on 8 cores, weights
replicated). Per core:
  Phase 1: X0 = im2col(x) @ W0 for all 511 patch positions as a dense conv
           matmul (contraction over channels, time-strided moving operand).
  Phase 2: 3-layer LSTM pipelined over 16-step blocks. Gate layout puts the
           4H=1024 gate dim on partitions as 8 chunks of 128 = (gate, half),
           gate order (g, i, f, o) so one Tanh op covers g and one Sigmoid op
           covers i,f,o. z lives in PSUM per block: bias via a one-hot K=8
           matmul (start=True), the t-parallel part (identity-matmul preload
           of X0 for layer 0 / blocked W@h_prev for layers 1,2) accumulates,
           then the per-step recurrent U@h matmuls accumulate in place.
Weights/data in bf16 ("bf16" mode) or bf16 hi+lo pairs ("split" mode, near
fp32 accuracy), fp32 PSUM accumulation everywhere.
"""

import sys

sys.path.insert(0, "/opt/trn_rl_repo")

import numpy as np
import ml_dtypes

import concourse.bass as bass
import concourse.mybir as mybir
import concourse.tile as tile
from concourse import bacc
from concourse.bass_utils import run_bass_kernel_spmd

F32 = mybir.dt.float32
BF16 = mybir.dt.bfloat16
AF = mybir.ActivationFunctionType

K, S, H, L = 8, 4, 256, 3
B, T, C = 32, 2048, 128
NCORES = 8
BPC = B // NCORES  # 4 batch rows per core
BLK = 16

# gate order in my chunk layout: (g, i, f, o); keras source order is (i, f, g, o)
SRC_GATE = [2, 0, 1, 3]  # my gate index -> source gate index

MODE = "bf16"  # "bf16" | "split"

_cache = {}


def _perm1024():
    # my column (c*128+m) with c=(g',hh) -> source column srcg*256 + hh*128 + m
    perm = np.empty(1024, np.int64)
    for c in range(8):
        gp, hh = c // 2, c % 2
        src = SRC_GATE[gp] * 256 + hh * 128
        perm[c * 128:(c + 1) * 128] = np.arange(src, src + 128)
    return perm


PERM = _perm1024()


def _bf(a):
    return a.astype(ml_dtypes.bfloat16)


def _split(a):
    hi = _bf(a)
    lo = _bf(a - hi.astype(np.float32))
    return hi, lo


def _w_arr(w):
    """[d_in, 4H] fp32 -> [128, kk*8*128] with stationary tiles at
    [:, (kk*8+c)*128 : +128]."""
    d_in = w.shape[0]
    kk = d_in // 128
    wr = w[:, PERM].reshape(kk, 128, 8, 128).transpose(1, 0, 2, 3)
    return np.ascontiguousarray(wr.reshape(128, kk * 8 * 128))


def _build(P, mode):
    """Build the SPMD Bass program for P patch steps."""
    nblocks = (P + BLK - 1) // BLK
    blocks = [(i * BLK, min(BLK, P - i * BLK)) for i in range(nblocks)]

    nc = bacc.Bacc("TRN2", target_bir_lowering=False, debug=False,
                   num_devices=NCORES)
    Teff = (P - 1) * S + K  # time extent actually read

    hilo = ["hi", "lo"] if mode == "split" else ["hi"]

    # ---- DRAM parameters ----
    xt_d = {s: nc.declare_dram_parameter(f"xt_{s}", [128, BPC, Teff], BF16,
                                         isOutput=False) for s in hilo}
    wt_d = {}
    for l in range(L):
        kkw = 8 if l == 0 else 2
        for s in hilo:
            wt_d[(l, "w", s)] = nc.declare_dram_parameter(
                f"w{l}_{s}", [128, kkw * 1024], BF16, isOutput=False)
            wt_d[(l, "u", s)] = nc.declare_dram_parameter(
                f"u{l}_{s}", [128, 2 * 1024], BF16, isOutput=False)
    b8_d = {s: nc.declare_dram_parameter(f"b8_{s}", [8, L * 128], BF16,
                                         isOutput=False) for s in hilo}
    oh_d = nc.declare_dram_parameter("oh", [8, 8, BLK, BPC], BF16,
                                     isOutput=False)
    idt = F32 if mode == "split" else BF16
    id_d = nc.declare_dram_parameter("idn", [128, 128], idt, isOutput=False)
    out_d = nc.declare_dram_parameter("out", [128, 2, P, BPC], F32,
                                      isOutput=True)

    x0dt = F32 if mode == "split" else BF16

    with tile.TileContext(nc) as tc:
        with (
            tc.tile_pool(name="consts", bufs=1) as consts,
            tc.tile_pool(name="x0pool", bufs=1) as x0pool,
            tc.tile_pool(name="gates", bufs=6) as gates,
            tc.tile_pool(name="hblk0", bufs=2) as hp0,
            tc.tile_pool(name="hblk1", bufs=2) as hp1,
            tc.tile_pool(name="hblk2", bufs=2) as hp2,
        ):
            hpools = [hp0, hp1, hp2]

            # ---- load constants ----
            xt = {}
            for s in hilo:
                t_ = consts.tile([128, BPC, Teff], BF16, name=f"xt{s}",
                                 tag=f"xt{s}")
                nc.sync.dma_start(out=t_[:], in_=xt_d[s].ap())
                xt[s] = t_
            wsb = {}
            for key, d in wt_d.items():
                t_ = consts.tile([128, d.shape[1]], BF16,
                                 name=f"w{key[0]}{key[1]}{key[2]}",
                                 tag=f"w{key[0]}{key[1]}{key[2]}")
                nc.sync.dma_start(out=t_[:], in_=d.ap())
                wsb[key] = t_
            b8 = {}
            for s in hilo:
                t_ = consts.tile([8, L * 128], BF16, name=f"b8{s}",
                                 tag=f"b8{s}")
                nc.sync.dma_start(out=t_[:], in_=b8_d[s].ap())
                b8[s] = t_
            oh = consts.tile([8, 8, BLK, BPC], BF16, tag="oh")
            nc.sync.dma_start(out=oh[:], in_=oh_d.ap())
            idn = consts.tile([128, 128], idt, tag="idn")
            nc.sync.dma_start(out=idn[:], in_=id_d.ap())

            x0t = x0pool.tile([128, 8, P, BPC], x0dt, tag="x0t")
            out_hist = consts.tile([128, 2, P, BPC], F32, tag="outh")

            zeros_h = consts.tile([128, 2, BPC], BF16, tag="zh")
            nc.vector.memset(zeros_h[:], 0.0)
            c_zero = consts.tile([128, 2, BPC], F32, tag="cz")
            nc.vector.memset(c_zero[:], 0.0)
            c_st = [[consts.tile([128, 2, BPC], F32, name=f"c{l}_{par}",
                                 tag=f"c{l}_{par}")
                     for par in range(2)] for l in range(L)]

            # ---- phase 1: X0 = im2col(x) @ W0 ----
            with tc.tile_pool(name="ph1", bufs=2, space="PSUM") as ph1:
                TC = 128  # time chunk
                ntc = (P + TC - 1) // TC
                for tci in range(ntc):
                    t0 = tci * TC
                    tcnt = min(TC, P - t0)
                    for c in range(8):
                        ps = ph1.tile([128, TC, BPC], F32, tag="ph1")
                        passes = []
                        for j in range(8):
                            if mode == "split":
                                passes += [(j, "hi", "hi"), (j, "hi", "lo"),
                                           (j, "lo", "hi")]
                            else:
                                passes += [(j, "hi", "hi")]
                        for pi, (j, ws, xs) in enumerate(passes):
                            mv = xt[xs][:, :, j + S * t0:
                                        j + S * (t0 + tcnt - 1) + 1: S]
                            mv = mv.rearrange("p n t -> p t n")
                            nc.tensor.matmul(
                                ps[:, :tcnt, :],
                                wsb[(0, "w", ws)][:, (j * 8 + c) * 128:
                                                  (j * 8 + c + 1) * 128],
                                mv,
                                start=(pi == 0), stop=(pi == len(passes) - 1),
                            )
                        nc.vector.tensor_copy(x0t[:, c, t0:t0 + tcnt, :],
                                              ps[:, :tcnt, :])

            # ---- phase 2 ----
            with (
                tc.tile_pool(name="zps0", bufs=2, space="PSUM") as zp0,
                tc.tile_pool(name="zps1", bufs=2, space="PSUM") as zp1,
                tc.tile_pool(name="zps2", bufs=2, space="PSUM") as zp2,
            ):
                zpools = [zp0, zp1, zp2]
                h_map = {}

                def process_block(l, b):
                    t0, cnt = blocks[b]
                    zt = zpools[l].tile([128, 8, BLK, BPC], F32, tag=f"z{l}")
                    # bias init (start=True over whole used range)
                    for si, s in enumerate(hilo):
                        nc.tensor.matmul(
                            zt[:, :, :cnt, :], b8[s][:, l * 128:(l + 1) * 128],
                            oh[:, :, :cnt, :],
                            start=(si == 0), stop=False)
                    if l == 0:
                        nc.tensor.matmul(zt[:, :, :cnt, :], idn[:],
                                         x0t[:, :, t0:t0 + cnt, :],
                                         start=False, stop=False)
                    else:
                        hb = h_map[(l - 1, b)]
                        for c in range(8):
                            for kk in range(2):
                                for ws in hilo:
                                    mvs = hilo if ws == "hi" else ["hi"]
                                    for xs in mvs:
                                        nc.tensor.matmul(
                                            zt[:, c, :cnt, :],
                                            wsb[(l, "w", ws)][:, (kk * 8 + c) * 128:
                                                              (kk * 8 + c + 1) * 128],
                                            hb[xs][:, kk, :cnt, :],
                                            start=False, stop=False)
                    hbl = {s: hpools[l].tile([128, 2, BLK, BPC], BF16,
                                             name=f"h{l}{s}_{b}",
                                             tag=f"h{l}{s}") for s in hilo}
                    h_map[(l, b)] = hbl
                    for tb in range(cnt):
                        t = t0 + tb
                        # recurrent U matmuls
                        for c in range(8):
                            last_c = (c == 7)
                            for kk in range(2):
                                passes = ([("hi", "hi"), ("hi", "lo"), ("lo", "hi")]
                                          if mode == "split" else [("hi", "hi")])
                                for pi, (ws, xs) in enumerate(passes):
                                    if t == 0:
                                        mv = zeros_h[:, kk, :]
                                    elif tb == 0:
                                        pb = h_map[(l, b - 1)]
                                        mv = pb[xs][:, kk, blocks[b - 1][1] - 1, :]
                                    else:
                                        mv = hbl[xs][:, kk, tb - 1, :]
                                    stop = (last_c and kk == 1
                                            and pi == len(passes) - 1)
                                    nc.tensor.matmul(
                                        zt[:, c, tb, :],
                                        wsb[(l, "u", ws)][:, (kk * 8 + c) * 128:
                                                          (kk * 8 + c + 1) * 128],
                                        mv, start=False, stop=stop)
                        # gates
                        th_g = gates.tile([128, 2, BPC], F32, tag="thg")
                        nc.scalar.activation(th_g[:], zt[:, 0:2, tb, :], AF.Tanh)
                        sg = gates.tile([128, 6, BPC], F32, tag="sg")
                        nc.scalar.activation(sg[:], zt[:, 2:8, tb, :], AF.Sigmoid)
                        cprev = c_st[l][(t + 1) % 2] if t > 0 else c_zero
                        q = gates.tile([128, 2, BPC], F32, tag="q")
                        nc.vector.tensor_mul(q[:], sg[:, 2:4, :], cprev[:])
                        p_ = gates.tile([128, 2, BPC], F32, tag="p")
                        nc.vector.tensor_mul(p_[:], sg[:, 0:2, :], th_g[:])
                        cn = c_st[l][t % 2]
                        nc.vector.tensor_add(cn[:], q[:], p_[:])
                        th_c = gates.tile([128, 2, BPC], F32, tag="thc")
                        nc.scalar.activation(th_c[:], cn[:], AF.Tanh)
                        if mode == "split":
                            hf = gates.tile([128, 2, BPC], F32, tag="hf")
                            nc.vector.tensor_mul(hf[:], sg[:, 4:6, :], th_c[:])
                            nc.vector.tensor_copy(hbl["hi"][:, :, tb, :], hf[:])
                            nc.vector.tensor_sub(hbl["lo"][:, :, tb, :], hf[:],
                                                 hbl["hi"][:, :, tb, :])
                            if l == 2:
                                nc.vector.tensor_copy(out_hist[:, :, t, :], hf[:])
                        else:
                            nc.vector.tensor_mul(hbl["hi"][:, :, tb, :],
                                                 sg[:, 4:6, :], th_c[:])
                            if l == 2:
                                nc.vector.tensor_mul(out_hist[:, :, t, :],
                                                     sg[:, 4:6, :], th_c[:])

                for sb in range(nblocks + L - 1):
                    for l in range(L):
                        b = sb - l
                        if 0 <= b < nblocks:
                            process_block(l, b)

            nc.sync.dma_start(out=out_d.ap(), in_=out_hist[:])

    nc.compile()
    return nc


def _prep_inputs(x, Ws, Us, bs, P, mode):
    """-> list of per-core input dicts."""
    Teff = (P - 1) * S + K
    hilo = ["hi", "lo"] if mode == "split" else ["hi"]

    base = {}
    for l in range(L):
        for nm, w in (("w", Ws[l]), ("u", Us[l])):
            arr = _w_arr(w)
            if mode == "split":
                hi, lo = _split(arr)
                base[f"{nm}{l}_hi"], base[f"{nm}{l}_lo"] = hi, lo
            else:
                base[f"{nm}{l}_hi"] = _bf(arr)
    b8f = np.concatenate([b[PERM].reshape(8, 128) for b in bs], axis=1)
    if mode == "split":
        base["b8_hi"], base["b8_lo"] = _split(b8f)
    else:
        base["b8_hi"] = _bf(b8f)
    ohm = np.zeros((8, 8, BLK, BPC), np.float32)
    for c in range(8):
        ohm[c, c] = 1.0
    base["oh"] = _bf(ohm)
    idn = np.eye(128, dtype=np.float32)
    base["idn"] = idn if mode == "split" else _bf(idn)

    in_maps = []
    for i in range(NCORES):
        m = dict(base)
        xs = x[i * BPC:(i + 1) * BPC, :Teff, :]  # [BPC, Teff, C]
        xtr = np.ascontiguousarray(xs.transpose(2, 0, 1))  # [128, BPC, Teff]
        if mode == "split":
            hi, lo = _split(xtr)
            m["xt_hi"] = hi
            m["xt_lo"] = lo
        else:
            m["xt_hi"] = _bf(xtr)
        in_maps.append(m)
    return in_maps


def _run(x, Ws, Us, bs, P=None, mode=None, trace=False):
    if P is None:
        P = (x.shape[1] - K) // S + 1
    if mode is None:
        mode = MODE
    key = (P, mode)
    if key not in _cache:
        _cache[key] = _build(P, mode)
    nc = _cache[key]
    in_maps = _prep_inputs(x, Ws, Us, bs, P, mode)
    res = run_bass_kernel_spmd(nc, in_maps, list(range(NCORES)), trace=trace)
    outs = []
    for i in range(NCORES):
        o = res.results[i]["out"].reshape(128, 2, P, BPC)
        # out[n, t, hh*128 + p] = o[p, hh, t, n]
        outs.append(np.ascontiguousarray(o.transpose(3, 2, 1, 0)
                                         .reshape(BPC, P, H)))
    return np.concatenate(outs, 0), res


def kernel(x, W0, U0, b0, W1, U1, b1, W2, U2, b2):
    x = np.asarray(x, np.float32)
    out, _ = _run(x,
                  [np.asarray(W0, np.float32), np.asarray(W1, np.float32),
                   np.asarray(W2, np.float32)],
                  [np.asarray(U0, np.float32), np.asarray(U1, np.float32),
                   np.asarray(U2, np.float32)],
                  [np.asarray(b0, np.float32), np.asarray(b1, np.float32),
                   np.asarray(b2, np.float32)])
    return out
